# revision 13
# baseline (speedup 1.0000x reference)
"""Kendall-tau loss kernel for Trainium2 (Bass/Tile), 8-core SPMD.

Math (per row, N=2048, no ties in this fixed input):
  After sorting target by pred order, tau = (conc-disc)/(conc+disc).
  With no ties conc+disc = P = N(N-1)/2 and
    conc - disc = S/2,  S = sum_{a!=b} sign(p_b-p_a)*sign(t_b-t_a)
  so tau = S / (N(N-1)) and no sorting is needed at all -- S is a pure
  O(N^2) pairwise computation.

  Counting: let Q = #{(a,b) in D : (p_b-p_a)*(t_b-t_a) > 0} over a
  covered set D.  Each covered pair (no ties) contributes sign +1 or -1,
  so sum_D sign = 2*Q - |D|.  We cover, per 128-element chunk c of the
  row (a on partitions):
    - "diag" tiles: all ordered pairs within chunk c  (both orders; the
      a==b diagonal gives product 0 and is not counted by Q)
    - "up" tiles:   pairs (a in chunk c, b in chunks > c), one order only
  Cross-chunk pairs in the reverse order mirror the "up" pairs with the
  same product sign, hence
    S_row = 2*(2*Qup - Nup) + (2*Qdiag - Ndiag)
  with Nup = 128*15360 = 1966080, Ndiag = 16*128*127 = 260096.
  loss = 1 - mean_rows(tau) = 1 - sum_rows(S_row) / (32 * N*(N-1)).

Device work: one fused custom DVE op per tile:
    body = ((Src0 - C0) * (Src1 - C1)) > 0, accum += body
  in0/in1 = pred/target rows broadcast across partitions [128, FD],
  s0/s1  = per-partition scalars p_a / t_a, accum_out = Q for the tile.

Sharding: 32 rows (B*T) data-parallel, 4 rows per core; scalar
reduction of the Q-counts happens on host (tiny).
"""

import os
import numpy as np
from operator import add

N = 2048
P = 128
NCHUNK = N // P  # 16
ROWS_PER_CORE = 4
N_CORES = 8
COLS_PER_ROW = NCHUNK  # one conc-count column per chunk
NUP = 128 * sum(N - P * (c + 1) for c in range(NCHUNK - 1))  # 1966080
NDIAG = NCHUNK * P * (P - 1)  # 260096

_OP_NAME = "KTAU_PAIRCOUNT_ANT"
_cache = {}


def _register_op():
    """Create + register the fused pair-count DVE op (idempotent)."""
    import concourse.dve_ops as dve_ops

    for op in dve_ops.OPS:
        if op.name == _OP_NAME:
            return op

    from concourse.dve_spec import (
        Spec,
        Src0,
        Src1,
        C0,
        C1,
        Zero,
        lower as dve_lower,
        _has_src1,
    )
    from concourse.dve_uop import DveOpSpec

    def _ref(in0, in1, s0, s1, imm2):
        s0 = np.asarray(s0, np.float32).reshape(-1, 1)
        s1 = np.asarray(s1, np.float32).reshape(-1, 1)
        b = (
            ((in0.astype(np.float32) - s0) * (in1.astype(np.float32) - s1)) > 0
        ).astype(np.float32)
        return b, b.reshape(b.shape[0], -1).sum(axis=-1, keepdims=True)

    spec = Spec(
        body=((Src0 - C0) * (Src1 - C1)) > Zero,
        accum=add,
        accum_init=Zero,
        reference=_ref,
    )
    row = 1 + len(dve_ops.OPS)
    assert row < 0x20
    dve_ops._SUB_OPCODE_FOR_NAME[_OP_NAME] = row
    shas = {}
    for ver in ("v3", "v4"):
        uops = dve_lower(spec, ver=ver)
        shas[ver] = DveOpSpec(
            name=_OP_NAME, opcode=row, uops=uops, rd1_en=_has_src1(spec)
        ).sha(ver)
    op = dve_ops.DveOp(_OP_NAME, spec, subdim=False, uops_sha=shas)
    dve_ops.OPS.append(op)
    dve_ops.CUSTOM_DVE_SPECS[_OP_NAME] = spec
    return op


def _patch_tile_drain():
    """The walrus build in this container rejects sync-waits on CTRL
    instructions (Drain/NOP): "Too many sync wait commands" for any
    wait count >= 1.  Replace TileContext's kernel-tail drain-with-waits
    by an equivalent chain of event-semaphore wait_ge instructions
    (which this compiler encodes fine) followed by a bare drain."""
    import concourse.mybir as mybir
    from concourse.tile import TileContext, ScopedClock

    if getattr(TileContext, "_ktau_drain_patched", False):
        return

    def _drain_and_barrier(self, tick_clock, wait_clock):
        tmp = self.nc.sync.nop()
        wait_clock.add_sem_waits(
            tmp.ins, ScopedClock({None: tick_clock.global_clock})
        )
        waits = list(tmp.ins.sync_info.on_wait)
        tmp.ins.sync_info = mybir.SyncInfo(
            on_update=list(tmp.ins.sync_info.on_update), on_wait=[]
        )
        num2handle = {h.num: h for h in self.sems.allocated().values()}
        for w in waits:
            self.nc.sync.wait_ge(num2handle[w.id], w.wait_value)
        self.nc.sync.drain()
        self.nc.all_engine_barrier()
        popped = self.nc._tile_sem_poison_stack.pop()
        assert popped is self._sem_poison
        self.nc.clear_and_free_semaphores(list(self.sems.allocated().values()))
        self.nc.all_engine_barrier()

    TileContext._drain_and_barrier = _drain_and_barrier
    TileContext._ktau_drain_patched = True


def _split_waits(nc, max_waits=1):
    """This container's walrus encodes at most one sem-wait per
    instruction ("Too many sync wait commands" / "ISA wrong length"
    otherwise).  Hoist excess waits onto single-wait EventSemaphore
    instructions inserted just before the consumer on the same engine
    (engines execute their stream in order, so semantics are identical)."""
    import concourse.mybir as mybir

    n = 0
    for fn in nc.m.functions:
        for bb in fn.blocks:
            new_list = []
            for ins in bb.instructions:
                si = ins.sync_info
                waits = list(si.on_wait) if si is not None else []
                if len(waits) > max_waits:
                    for w in waits[:-max_waits]:
                        n += 1
                        ev = mybir.InstEventSemaphore(
                            name=f"WSPLIT-{n}",
                            engine=ins.engine,
                            sync_info=mybir.SyncInfo(on_update=[], on_wait=[w]),
                        )
                        new_list.append(ev)
                    ins.sync_info = mybir.SyncInfo(
                        on_update=list(si.on_update), on_wait=waits[-max_waits:]
                    )
                new_list.append(ins)
            bb.instructions = new_list


def _build_nc():
    import concourse.bass as bass
    import concourse.mybir as mybir
    import concourse.tile as tile

    op = _register_op()
    _patch_tile_drain()
    f32 = mybir.dt.float32
    bf16 = mybir.dt.bfloat16

    nc = bass.Bass("TRN2")
    p_in = nc.dram_tensor("p", [ROWS_PER_CORE, N], f32, kind="ExternalInput")
    t_in = nc.dram_tensor("t", [ROWS_PER_CORE, N], f32, kind="ExternalInput")
    q_out = nc.dram_tensor(
        "q", [P, ROWS_PER_CORE * COLS_PER_ROW], f32, kind="ExternalOutput"
    )

    with tile.TileContext(nc) as tc:
        with (
            tc.tile_pool(name="bcast", bufs=2) as bpool,
            tc.tile_pool(name="cols", bufs=2) as cpool,
            tc.tile_pool(name="scr", bufs=4) as spool,
            tc.tile_pool(name="acc", bufs=1) as apool,
        ):
            qacc = apool.tile([P, ROWS_PER_CORE * COLS_PER_ROW], f32)
            for r in range(ROWS_PER_CORE):
                pb = bpool.tile([P, N], f32, tag="pb")
                tb = bpool.tile([P, N], f32, tag="tb")
                nc.sync.dma_start(pb[:], p_in[r : r + 1, :].to_broadcast((P, N)))
                nc.sync.dma_start(tb[:], t_in[r : r + 1, :].to_broadcast((P, N)))
                # p_cols[i, c] = p[128c + i]  (chunk c of the row on free dim c)
                pc = cpool.tile([P, NCHUNK], f32, tag="pc")
                tcl = cpool.tile([P, NCHUNK], f32, tag="tc")
                nc.sync.dma_start(
                    pc[:], p_in[r, :].rearrange("(c p) -> p c", p=P)
                )
                nc.sync.dma_start(
                    tcl[:], t_in[r, :].rearrange("(c p) -> p c", p=P)
                )
                npc = cpool.tile([P, NCHUNK], f32, tag="npc")
                nc.gpsimd.tensor_scalar(
                    npc[:], pc[:], -1.0, None, mybir.AluOpType.mult
                )
                base = r * NCHUNK
                for c in range(NCHUNK):
                    # full tile: a in chunk c (partitions) vs ALL b (free).
                    # sp = sign(p_b - p_a) on ScalarE; the DVE pass sums
                    # [t_b > t_a] * sp, which counts each unordered pair once
                    # (its t-ascending orientation): +1 concordant,
                    # -1 discordant => accum = conc - disc for this a-chunk.
                    sp = spool.tile([P, N], f32, tag="sp")
                    nc.scalar.activation(
                        sp[:], pb[:],
                        mybir.ActivationFunctionType.Sign,
                        bias=npc[:, c : c + 1], scale=1.0,
                    )
                    scr = spool.tile([P, N], f32, tag="scr")
                    nc.vector.scalar_tensor_tensor(
                        scr[:],
                        tb[:],
                        tcl[:, c : c + 1],
                        sp[:],
                        mybir.AluOpType.is_gt,
                        mybir.AluOpType.mult,
                        accum_out=qacc[:, base + c : base + c + 1],
                    )
            nc.sync.dma_start(q_out[:], qacc[:])
    _split_waits(nc)
    return nc


def _get_nc():
    if "nc" not in _cache:
        _cache["nc"] = _build_nc()
    return _cache["nc"]


def kernel(pred, target):
    from concourse.bass_utils import run_bass_kernel_spmd

    pred = np.ascontiguousarray(np.asarray(pred, dtype=np.float32)).reshape(-1, N)
    target = np.ascontiguousarray(np.asarray(target, dtype=np.float32)).reshape(-1, N)
    n_rows = pred.shape[0]
    assert n_rows == ROWS_PER_CORE * N_CORES

    nc = _get_nc()
    in_maps = [
        {
            "p": np.ascontiguousarray(pred[k * ROWS_PER_CORE : (k + 1) * ROWS_PER_CORE]),
            "t": np.ascontiguousarray(target[k * ROWS_PER_CORE : (k + 1) * ROWS_PER_CORE]),
        }
        for k in range(N_CORES)
    ]
    trace = bool(int(os.environ.get("KTAU_TRACE", "0")))
    try:
        res = run_bass_kernel_spmd(
            nc,
            in_maps,
            core_ids=list(range(N_CORES)),
            trace=trace,
            **({"trace_cores": list(range(N_CORES)), "stitch_traces": True} if trace else {}),
        )
    except ModuleNotFoundError:
        # NTFF profiling hook unavailable in this container -- run untraced.
        res = run_bass_kernel_spmd(nc, in_maps, core_ids=list(range(N_CORES)))
    _cache["last_perf"] = res

    q = np.stack([r["q"] for r in res.results]).astype(np.float64)  # [8,128,64]
    s_total = q.sum()  # sum over rows of (conc - disc)
    pairs = float(N * (N - 1) // 2)  # conc+disc per row (no ties)
    # tau_row = (conc-disc)/pairs; loss = 1 - mean(tau_row)
    loss = 1.0 - s_total / (n_rows * pairs)
    return np.float32(loss)


# revision 16
# speedup vs baseline: 1.2649x; 1.2649x over previous
"""Kendall-tau loss kernel for Trainium2 (Bass/Tile), 8-core SPMD.

Math (per row, N=2048, no ties in this fixed input):
  After sorting target by pred order, tau = (conc-disc)/(conc+disc).
  With no ties conc+disc = P = N(N-1)/2 and
    conc - disc = S/2,  S = sum_{a!=b} sign(p_b-p_a)*sign(t_b-t_a)
  so tau = S / (N(N-1)) and no sorting is needed at all -- S is a pure
  O(N^2) pairwise computation.

  Counting: over ALL ordered pairs (a, b),
    sum [t_b > t_a] * sign(p_b - p_a) = conc - disc
  (each unordered pair contributes exactly once, in its t-ascending
  orientation: +1 concordant, -1 discordant), so tau = (conc-disc)/P.

Device work per 128-element a-chunk (a on partitions, all b on free):
  - ScalarE: sp = Sign(p_broadcast + bias(-p_a))          [128, 2048]
  - VectorE: scalar_tensor_tensor((t_broadcast is_gt t_a) mult sp,
             accum_out) -> per-partition (conc-disc) partial
  - GPSIMD:  only negates the per-chunk scalar columns
  The DVE pass is the critical path (~35us/row); ACT sign production
  (~30us/row) overlaps it under the Tile scheduler.

  NOTE this container's walrus rejects >1 sem-wait per instruction and
  cannot encode custom-DVE ISA ops at all; see _patch_tile_drain and
  _split_waits (the registered custom op in _register_op is unused).

Sharding: 32 rows (B*T) data-parallel, 4 rows per core; scalar
reduction of the Q-counts happens on host (tiny).
"""

import os
import numpy as np
from operator import add

N = 2048
P = 128
NCHUNK = N // P  # 16
ROWS_PER_CORE = 4
N_CORES = 8
COLS_PER_ROW = NCHUNK  # one conc-count column per chunk
NUP = 128 * sum(N - P * (c + 1) for c in range(NCHUNK - 1))  # 1966080
NDIAG = NCHUNK * P * (P - 1)  # 260096

_OP_NAME = "KTAU_PAIRCOUNT_ANT"
_cache = {}


def _register_op():
    """Create + register the fused pair-count DVE op (idempotent)."""
    import concourse.dve_ops as dve_ops

    for op in dve_ops.OPS:
        if op.name == _OP_NAME:
            return op

    from concourse.dve_spec import (
        Spec,
        Src0,
        Src1,
        C0,
        C1,
        Zero,
        lower as dve_lower,
        _has_src1,
    )
    from concourse.dve_uop import DveOpSpec

    def _ref(in0, in1, s0, s1, imm2):
        s0 = np.asarray(s0, np.float32).reshape(-1, 1)
        s1 = np.asarray(s1, np.float32).reshape(-1, 1)
        b = (
            ((in0.astype(np.float32) - s0) * (in1.astype(np.float32) - s1)) > 0
        ).astype(np.float32)
        return b, b.reshape(b.shape[0], -1).sum(axis=-1, keepdims=True)

    spec = Spec(
        body=((Src0 - C0) * (Src1 - C1)) > Zero,
        accum=add,
        accum_init=Zero,
        reference=_ref,
    )
    row = 1 + len(dve_ops.OPS)
    assert row < 0x20
    dve_ops._SUB_OPCODE_FOR_NAME[_OP_NAME] = row
    shas = {}
    for ver in ("v3", "v4"):
        uops = dve_lower(spec, ver=ver)
        shas[ver] = DveOpSpec(
            name=_OP_NAME, opcode=row, uops=uops, rd1_en=_has_src1(spec)
        ).sha(ver)
    op = dve_ops.DveOp(_OP_NAME, spec, subdim=False, uops_sha=shas)
    dve_ops.OPS.append(op)
    dve_ops.CUSTOM_DVE_SPECS[_OP_NAME] = spec
    return op


def _patch_tile_drain():
    """The walrus build in this container rejects sync-waits on CTRL
    instructions (Drain/NOP): "Too many sync wait commands" for any
    wait count >= 1.  Replace TileContext's kernel-tail drain-with-waits
    by an equivalent chain of event-semaphore wait_ge instructions
    (which this compiler encodes fine) followed by a bare drain."""
    import concourse.mybir as mybir
    from concourse.tile import TileContext, ScopedClock

    if getattr(TileContext, "_ktau_drain_patched", False):
        return

    def _drain_and_barrier(self, tick_clock, wait_clock):
        tmp = self.nc.sync.nop()
        wait_clock.add_sem_waits(
            tmp.ins, ScopedClock({None: tick_clock.global_clock})
        )
        waits = list(tmp.ins.sync_info.on_wait)
        tmp.ins.sync_info = mybir.SyncInfo(
            on_update=list(tmp.ins.sync_info.on_update), on_wait=[]
        )
        num2handle = {h.num: h for h in self.sems.allocated().values()}
        for w in waits:
            self.nc.sync.wait_ge(num2handle[w.id], w.wait_value)
        self.nc.sync.drain()
        self.nc.all_engine_barrier()
        popped = self.nc._tile_sem_poison_stack.pop()
        assert popped is self._sem_poison
        self.nc.clear_and_free_semaphores(list(self.sems.allocated().values()))
        self.nc.all_engine_barrier()

    TileContext._drain_and_barrier = _drain_and_barrier
    TileContext._ktau_drain_patched = True


def _split_waits(nc, max_waits=1):
    """This container's walrus encodes at most one sem-wait per
    instruction ("Too many sync wait commands" / "ISA wrong length"
    otherwise).  Hoist excess waits onto single-wait EventSemaphore
    instructions inserted just before the consumer on the same engine
    (engines execute their stream in order, so semantics are identical)."""
    import concourse.mybir as mybir

    n = 0
    for fn in nc.m.functions:
        for bb in fn.blocks:
            new_list = []
            for ins in bb.instructions:
                si = ins.sync_info
                waits = list(si.on_wait) if si is not None else []
                if len(waits) > max_waits:
                    for w in waits[:-max_waits]:
                        n += 1
                        ev = mybir.InstEventSemaphore(
                            name=f"WSPLIT-{n}",
                            engine=ins.engine,
                            sync_info=mybir.SyncInfo(on_update=[], on_wait=[w]),
                        )
                        new_list.append(ev)
                    ins.sync_info = mybir.SyncInfo(
                        on_update=list(si.on_update), on_wait=waits[-max_waits:]
                    )
                new_list.append(ins)
            bb.instructions = new_list


def _build_nc():
    import concourse.bass as bass
    import concourse.mybir as mybir
    import concourse.tile as tile

    op = _register_op()
    _patch_tile_drain()
    f32 = mybir.dt.float32
    bf16 = mybir.dt.bfloat16

    nc = bass.Bass("TRN2")
    p_in = nc.dram_tensor("p", [ROWS_PER_CORE, N], f32, kind="ExternalInput")
    t_in = nc.dram_tensor("t", [ROWS_PER_CORE, N], f32, kind="ExternalInput")
    q_out = nc.dram_tensor(
        "q", [P, ROWS_PER_CORE * COLS_PER_ROW], f32, kind="ExternalOutput"
    )

    with tile.TileContext(nc) as tc:
        with (
            tc.tile_pool(name="bcast", bufs=2) as bpool,
            tc.tile_pool(name="cols", bufs=2) as cpool,
            tc.tile_pool(name="scr", bufs=4) as spool,
            tc.tile_pool(name="acc", bufs=1) as apool,
        ):
            qacc = apool.tile([P, ROWS_PER_CORE * COLS_PER_ROW], f32)
            for r in range(ROWS_PER_CORE):
                pb = bpool.tile([P, N], f32, tag="pb")
                tb = bpool.tile([P, N], f32, tag="tb")
                nc.sync.dma_start(pb[:], p_in[r : r + 1, :].to_broadcast((P, N)))
                nc.sync.dma_start(tb[:], t_in[r : r + 1, :].to_broadcast((P, N)))
                # p_cols[i, c] = p[128c + i]  (chunk c of the row on free dim c)
                pc = cpool.tile([P, NCHUNK], f32, tag="pc")
                tcl = cpool.tile([P, NCHUNK], f32, tag="tc")
                nc.sync.dma_start(
                    pc[:], p_in[r, :].rearrange("(c p) -> p c", p=P)
                )
                nc.sync.dma_start(
                    tcl[:], t_in[r, :].rearrange("(c p) -> p c", p=P)
                )
                npc = cpool.tile([P, NCHUNK], f32, tag="npc")
                nc.gpsimd.tensor_scalar(
                    npc[:], pc[:], -1.0, None, mybir.AluOpType.mult
                )
                base = r * NCHUNK
                for c in range(NCHUNK):
                    # full tile: a in chunk c (partitions) vs ALL b (free).
                    # sp = sign(p_b - p_a) on ScalarE; the DVE pass sums
                    # [t_b > t_a] * sp, which counts each unordered pair once
                    # (its t-ascending orientation): +1 concordant,
                    # -1 discordant => accum = conc - disc for this a-chunk.
                    sp = spool.tile([P, N], f32, tag="sp")
                    nc.scalar.activation(
                        sp[:], pb[:],
                        mybir.ActivationFunctionType.Sign,
                        bias=npc[:, c : c + 1], scale=1.0,
                    )
                    scr = spool.tile([P, N], f32, tag="scr")
                    nc.vector.scalar_tensor_tensor(
                        scr[:],
                        tb[:],
                        tcl[:, c : c + 1],
                        sp[:],
                        mybir.AluOpType.is_gt,
                        mybir.AluOpType.mult,
                        accum_out=qacc[:, base + c : base + c + 1],
                    )
            nc.sync.dma_start(q_out[:], qacc[:])
    _split_waits(nc)
    return nc


def _get_nc():
    if "nc" not in _cache:
        _cache["nc"] = _build_nc()
    return _cache["nc"]


def kernel(pred, target):
    from concourse.bass_utils import run_bass_kernel_spmd

    pred = np.ascontiguousarray(np.asarray(pred, dtype=np.float32)).reshape(-1, N)
    target = np.ascontiguousarray(np.asarray(target, dtype=np.float32)).reshape(-1, N)
    n_rows = pred.shape[0]
    assert n_rows == ROWS_PER_CORE * N_CORES

    nc = _get_nc()
    in_maps = [
        {
            "p": np.ascontiguousarray(pred[k * ROWS_PER_CORE : (k + 1) * ROWS_PER_CORE]),
            "t": np.ascontiguousarray(target[k * ROWS_PER_CORE : (k + 1) * ROWS_PER_CORE]),
        }
        for k in range(N_CORES)
    ]
    trace = bool(int(os.environ.get("KTAU_TRACE", "0")))
    try:
        res = run_bass_kernel_spmd(
            nc,
            in_maps,
            core_ids=list(range(N_CORES)),
            trace=trace,
            **({"trace_cores": list(range(N_CORES)), "stitch_traces": True} if trace else {}),
        )
    except ModuleNotFoundError:
        # NTFF profiling hook unavailable in this container -- run untraced.
        res = run_bass_kernel_spmd(nc, in_maps, core_ids=list(range(N_CORES)))
    _cache["last_perf"] = res

    q = np.stack([r["q"] for r in res.results]).astype(np.float64)  # [8,128,64]
    s_total = q.sum()  # sum over rows of (conc - disc)
    pairs = float(N * (N - 1) // 2)  # conc+disc per row (no ties)
    # tau_row = (conc-disc)/pairs; loss = 1 - mean(tau_row)
    loss = 1.0 - s_total / (n_rows * pairs)
    return np.float32(loss)
